# revision 83
# baseline (speedup 1.0000x reference)
"""Trainium2 Bass kernel for nn_AttentionBlock (B=8, S=1024, H=1024, 16 heads).

Strategy: data parallel — one batch element per NeuronCore (8 cores).

All four matmul families run in fp8e4 with DoubleRow perf mode (2 fp8
weights per PE cell => half the cycles per output element):

  QKV projections   lhsT = W chunk-pairs [128,2,128], rhs = x [128,2,512]
  scores            d-dim (64) split as [32 partitions x 2 planes]:
                    lhsT = k32 [32,2,128], rhs = q32 [32,2,512]
  probs @ v         j-tile pairs: lhsT = v [128,2,65], rhs = eT [128,2,512]
  out projection    lhsT = ctx head-pairs [64,2,128], rhs = Wo [64,2,512]

The softmax exp is the serial bottleneck (16.8M elements through one
engine); it is split between ScalarE (true exp, fp8 out) and the DVE
(Schraudolph trick: y8 = rne(A*s + B) written as int8 IS the fp8e4 bit
pattern of ~exp(s); rounding is RNE with saturation, so the -1e4 mask
bias saturates to -128 = -0 = exact zero probability). Scores psum runs
as a 3-deep ring so the exp->scores->exp slot hop hides behind two
in-flight exps.

Softmax normalization: denominators ride as a 65th ones-column through
the ctx matmul, accumulated per [65, 512] ih-half round (1 psum bank
each); DVE reciprocal reads the PSUM row directly; the row is broadcast
across partitions via a DRAM bounce (stride-0 DMA is DRAM-source-only);
one DVE multiply writes normalized fp8 ctx into [64, 2, S] head-pair
tiles (all heads' ctx rows at partitions 0:64 — no partition shifts).
Rounds are software-pipelined two heads deep, injected between the next
head's exps so the in-order engine queues never stall on the chain.

Phase C: out-projection runs as an hp0..6 partial (gated only by the
14th head, evacuated to bf16 by ScalarE during the last norm chains)
plus a short finish (hp7 + identity-matmul adds of the partial and the
residual). LayerNorm stats read PSUM directly; the normalize step is a
single ScalarE Identity with per-partition scale=rstd, bias=-mean*rstd.

Scaling ladder (all powers of two, folded where free):
  W{q,k,v} x16 on host -> projections epilogue x(1/16)  (q,k at true scale)
  v epilogue x2 => v_sb = 32*v_true; ones-column stays 1.0
  ctxT = ctx_ps * (1/denominator) = 32*ctx_true   (fp8-friendly)
  Wo x64 on host => att_ps = 2048*att_true
  xres = 2048*(query + Wo@bv + bo) on host (bf16); LayerNorm is
  scale-invariant (eps scaled by 2048^2), so the output is exact up to
  rounding. Output returned as bf16, upcast on host.
"""

import numpy as np
import ml_dtypes

import concourse.bass as bass
import concourse.mybir as mybir
import concourse.tile as tile
from concourse import bacc
from concourse.bass_utils import run_bass_kernel_spmd

BF16 = mybir.dt.bfloat16
FP32 = mybir.dt.float32
FP8 = mybir.dt.float8e4
I8 = mybir.dt.int8

B, S, H = 8, 1024, 1024
NH, DH = 16, 64
P = 128
KO = H // P          # 8 feature chunks of 128
ST = S // P          # 8 s-tiles
LN_EPS = 1e-7

W_SCALE = 16.0       # host premultiply on Wq/Wk/Wv
V_SCALE = 32.0       # v_sb = V_SCALE * v_true
WO_SCALE = 64.0      # host premultiply on Wo
RES_SCALE = V_SCALE * WO_SCALE  # att_ps and xres scale (2048)

# Schraudolph exp->fp8e4: y8 = rne(EXPA*(s) + 56 + EXPD)
EXPA = 8.0 / np.log(2.0)
EXPD = -0.46
SCORE_SCALE = 0.125  # 1/sqrt(DH)

# exp tiles handled by DVE (Schraudolph) instead of ScalarE: predicate on
# the global (head*8+j) index
def DVE_PRED(i):
    # denser Schraudolph share while the DVE has no normalization work yet
    # (head 0: 4 tiles, heads 1-2: 3), steady 2/head after
    h, j = i // 8, i % 8
    if h == 0:
        return j in (1, 3, 5, 7)
    if h in (1, 2):
        return j in (1, 4, 6)
    return j in (2, 5)
NORM_Q1 = "sync"  # queue for the rec-row -> DRAM hop
NORM_Q2 = "sync"  # queue for the DRAM -> bcast hop
PSS_BUFS = 3
PSC_BUFS = 2
TAIL_H = 99
EPI_MODE = 0
ET_BUFS = 6
NORMP_BUFS = 4
DRAMP_BUFS = 2
LOAD_STEP = 2
PSO_BUFS = 3
LNP_BUFS = 3
DVE_ADD_FROM = 8
PARTIAL_HPS = 5

_nbf8 = ml_dtypes.float8_e4m3
DR = mybir.MatmulPerfMode.DoubleRow


def _build_program(ln_affine=True):
    nc = bacc.Bacc(
        "TRN2",
        target_bir_lowering=False,
        debug=False,
        enable_asserts=False,
        num_devices=1,
    )

    def din(name, shape, dt):
        return nc.dram_tensor(name, shape, dt, kind="ExternalInput").ap()

    xq8 = din("xq8", [H, S], FP8)            # query.T fp8
    xk8 = din("xk8", [H, S], FP8)
    xv8 = din("xv8", [H, S], FP8)
    wq8 = din("wq8", [H, H], FP8)            # (Wq*16).T  [h, o]
    wk8 = din("wk8", [H, H], FP8)
    wv8 = din("wv8", [H, H], FP8)
    wo8 = din("wo8", [DH, NH, H], FP8)       # (Wo*64).T as [d, h, o]
    bq = din("bq", [P, KO], FP32)            # bq tiled [pi, po]
    maskb = din("maskb", [P, ST], FP32)      # -1e4*(1-mask) tiled
    sbias = din("sbias", [P, ST], FP32)      # EXPA*maskb + 56 + EXPD
    xres = din("xres", [S, H], BF16)         # 2048*(query + Wo@bv + bo)
    ident = din("ident", [P, P], BF16)       # identity (residual-add matmul)
    if ln_affine:
        lng = din("lng", [P, H], FP32)
        lnb = din("lnb", [P, H], FP32)
    out = nc.dram_tensor("out", [S, H], BF16, kind="ExternalOutput").ap()

    def kchunks(ap2d):
        return ap2d.rearrange("(po pi) f -> pi po f", pi=P)

    with tile.TileContext(nc) as tc:
        with (
            tc.tile_pool(name="wpool", bufs=1) as wpool,
            tc.tile_pool(name="xpool", bufs=1) as xpool,
            tc.tile_pool(name="acts", bufs=1) as acts,
            tc.tile_pool(name="small", bufs=1) as small,
            tc.tile_pool(name="eT", bufs=ET_BUFS) as eTp,
            tc.tile_pool(name="normp", bufs=NORMP_BUFS) as normp,
            tc.tile_pool(name="lnp", bufs=LNP_BUFS) as lnp,
            tc.tile_pool(name="dramp", bufs=DRAMP_BUFS, space="DRAM") as dramp,
        ):
            # ---- constants ----
            bq_sb = small.tile([P, KO], FP32, tag="bq")
            nc.sync.dma_start(bq_sb[:], bq)
            maskb_sb = small.tile([P, ST], FP32, tag="maskb")
            nc.sync.dma_start(maskb_sb[:], maskb)
            sbias_sb = small.tile([P, ST], FP32, tag="sbias")
            nc.sync.dma_start(sbias_sb[:], sbias)
            eps_sb = small.tile([P, 1], FP32, tag="eps")
            nc.vector.memset(eps_sb[:], LN_EPS * RES_SCALE * RES_SCALE)

            # persistent activations
            qT8 = acts.tile([P, KO, S], FP8, tag="qT8")      # [o, s]
            kT8 = acts.tile([P, KO, S], FP8, tag="kT8")
            # DR32 scores layout: head h lives at partitions 32*(h%2)..+32,
            # slot h//2, planes = d_hi. [dlo(2 groups), h//2, dhi, s]
            q32 = acts.tile([2 * 32, NH // 2, 2, S], FP8, tag="q32")
            k32 = acts.tile([2 * 32, NH // 2, 2, S], FP8, tag="k32")
            v_sb = acts.tile([P, ST, NH * (DH + 1)], FP8, tag="v")
            vv = v_sb[:].rearrange("p s (h e) -> p s h e", e=DH + 1)
            nc.gpsimd.memset(v_sb[:], 1.0)  # ones-columns; data overwritten
            # ctxT as 8 head-pair tiles [64, 2, S] (plane = head parity);
            # every head's 64 ctx rows live at partitions 0:64, so the PE
            # 65-row output (base 0) needs no partition shift, ever.
            ctxp = [
                acts.tile([DH, 2, S], FP8, tag=f"ctxp{i}", name=f"ctxp{i}")
                for i in range(NH // 2)
            ]
            # ---- phase A: projections (fp8 DoubleRow) ----
            def rearrange32(src, dst, half):
                # [o=h*64+d on partitions] -> [32g+dlo, h//2, dhi, s].
                # chunk k holds heads 2k (parts 0:64) / 2k+1 (64:128);
                # head h -> partition group h%2, slot h//2 = k.
                ksl = slice(half * 4, half * 4 + 4)
                for par in range(2):
                    for dhi in range(2):
                        nc.sync.dma_start(
                            dst[32 * par : 32 * par + 32, ksl, dhi, :],
                            src[par * 64 + dhi * 32 : par * 64 + dhi * 32 + 32, ksl, :],
                        )

            with (
                tc.tile_pool(name="psA", bufs=2, space="PSUM") as psA,
                tc.tile_pool(name="awp", bufs=3) as awp,
                tc.tile_pool(name="axp", bufs=3) as axp,
            ):
                def load_wx(wap, xap, nm):
                    # 2-chunk pieces so the first k-pair matmuls start after
                    # ~1/4 of the bytes instead of the whole tensor
                    w_sb = awp.tile([P, KO, H], FP8, tag="w", name=f"w_{nm}")
                    x_sb = axp.tile([P, KO, S], FP8, tag="x", name=f"x_{nm}")
                    for k in range(0, KO, LOAD_STEP):
                        nc.sync.dma_start(
                            w_sb[:, k : k + LOAD_STEP], kchunks(wap)[:, k : k + LOAD_STEP]
                        )
                        nc.sync.dma_start(
                            x_sb[:, k : k + LOAD_STEP], kchunks(xap)[:, k : k + LOAD_STEP]
                        )
                    return w_sb, x_sb

                def qk_proj(w_sb, x_sb, dst, half, bias):
                    for ot in range(half * 4, half * 4 + 4):
                        ps = psA.tile([P, S], FP32, tag="psA")
                        for sh in range(2):
                            for k in range(0, KO, 2):
                                nc.tensor.matmul(
                                    ps[:, sh * 512 : (sh + 1) * 512],
                                    lhsT=w_sb[:, k : k + 2, ot * P : (ot + 1) * P],
                                    rhs=x_sb[:, k : k + 2, sh * 512 : (sh + 1) * 512],
                                    start=(k == 0),
                                    stop=(k == KO - 2),
                                    perf_mode=DR,
                                )
                        # epilogues alternate ACT/DVE (both idle-ish in A)
                        use_act = (ot % 2 == 0) if EPI_MODE == 0 else (EPI_MODE == 2)
                        if use_act:
                            nc.scalar.activation(
                                out=dst[:, ot, :], in_=ps[:],
                                func=mybir.ActivationFunctionType.Identity,
                                scale=1.0 / W_SCALE,
                                bias=bq_sb[:, ot : ot + 1] if bias else 0.0,
                            )
                        else:
                            nc.vector.tensor_scalar(
                                out=dst[:, ot, :], in0=ps[:],
                                scalar1=1.0 / W_SCALE,
                                scalar2=bq_sb[:, ot : ot + 1] if bias else 0.0,
                                op0=mybir.AluOpType.mult,
                                op1=mybir.AluOpType.add,
                            )

                wq_sb, xq_sb = load_wx(wq8, xq8, "q")
                wk_sb, xk_sb = load_wx(wk8, xk8, "k")
                wv_sb, xv_sb = load_wx(wv8, xv8, "v")
                qk_proj(wq_sb, xq_sb, qT8, 0, bias=True)
                rearrange32(qT8, q32, 0)
                qk_proj(wk_sb, xk_sb, kT8, 0, bias=False)
                rearrange32(kT8, k32, 0)

                # v projection (natural layout with 65-wide head slots)
                for st in range(ST):
                    ps = psA.tile([P, H], FP32, tag="psA")
                    for oh in range(2):
                        for k in range(0, KO, 2):
                            nc.tensor.matmul(
                                ps[:, oh * 512 : (oh + 1) * 512],
                                lhsT=xv_sb[:, k : k + 2, st * P : (st + 1) * P],
                                rhs=wv_sb[:, k : k + 2, oh * 512 : (oh + 1) * 512],
                                start=(k == 0),
                                stop=(k == KO - 2),
                                perf_mode=DR,
                            )
                    src = ps[:].rearrange("p (h e) -> p h e", e=DH)
                    nc.scalar.activation(
                        out=vv[:, st, :, 0:DH], in_=src,
                        func=mybir.ActivationFunctionType.Identity,
                        scale=V_SCALE / W_SCALE,
                    )

                qk_proj(wq_sb, xq_sb, qT8, 1, bias=True)
                rearrange32(qT8, q32, 1)
                qk_proj(wk_sb, xk_sb, kT8, 1, bias=False)
                rearrange32(kT8, k32, 1)

            # Wo load during phase B
            wo_sb = wpool.tile([DH, NH, H], FP8, tag="wo")
            nc.sync.dma_start(wo_sb[:], wo8)

            # ---- phase B: attention, one head at a time ----
            with (
                tc.tile_pool(name="psS", bufs=PSS_BUFS, space="PSUM") as psS,
                tc.tile_pool(name="psC", bufs=PSC_BUFS, space="PSUM") as psC,
            ):
                def emit_scores_exps(h, mid_cbs=()):
                    # mid_cbs: {j: callback} — DVE norm-ops for earlier heads
                    # injected between this head's exps so the in-order DVE
                    # queue never runs a 2.4us norm burst that stalls ACT
                    eTs = []
                    g = slice(32 * (h % 2), 32 * (h % 2) + 32)
                    for j in range(ST):
                        if j in mid_cbs:
                            mid_cbs[j]()
                        jj = j % 2
                        if jj == 0:
                            eT = eTp.tile(
                                [P, 2, S], FP8, tag="eT", name=f"eT{h}_{j//2}"
                            )
                            eTs.append(eT)
                        eT = eTs[-1]
                        sc = psS.tile([P, S], FP32, tag="psS", name=f"sc{h}_{j}")
                        for ih in range(2):
                            nc.tensor.matmul(
                                sc[:, ih * 512 : (ih + 1) * 512],
                                lhsT=k32[g, h // 2, :, j * P : (j + 1) * P],
                                rhs=q32[g, h // 2, :, ih * 512 : (ih + 1) * 512],
                                start=True,
                                stop=True,
                                perf_mode=DR,
                            )
                        if DVE_PRED(h * ST + j):
                            # Schraudolph exp on DVE: int8 bits are fp8e4
                            nc.vector.tensor_scalar(
                                out=eT[:, jj, :].bitcast(I8),
                                in0=sc[:],
                                scalar1=float(EXPA * SCORE_SCALE),
                                scalar2=sbias_sb[:, j : j + 1],
                                op0=mybir.AluOpType.mult,
                                op1=mybir.AluOpType.add,
                            )
                        else:
                            nc.scalar.activation(
                                out=eT[:, jj, :], in_=sc[:],
                                func=mybir.ActivationFunctionType.Exp,
                                scale=SCORE_SCALE,
                                bias=maskb_sb[:, j : j + 1],
                            )
                    return eTs

                def emit_ctx_recip(h, ih, eTs):
                    # one [65, 512] ih-half accumulator (single psum bank)
                    cps = psC.tile(
                        [P, 512], FP32, tag="psC", name=f"ctxu{h}_{ih}"
                    )
                    for jp in range(ST // 2):
                        nc.tensor.matmul(
                            cps[0 : DH + 1, :],
                            lhsT=vv[:, 2 * jp : 2 * jp + 2, h, :],
                            rhs=eTs[jp][:, :, ih * 512 : (ih + 1) * 512],
                            start=(jp == 0),
                            stop=(jp == ST // 2 - 1),
                            perf_mode=DR,
                        )
                    # rec row -> DRAM -> stride-0 broadcast across partitions
                    rec = normp.tile(
                        [DH + 1, 512], BF16, tag="rec", name=f"rec{h}_{ih}"
                    )
                    with nc.allow_low_precision(reason="softmax denom recip; 2e-2 tol"):
                        nc.vector.reciprocal(
                            out=rec[DH : DH + 1, :], in_=cps[DH : DH + 1, :]
                        )
                    scr = dramp.tile([1, 512], BF16, tag="dscr", name=f"ds{h}_{ih}")
                    q1 = "gpsimd" if h >= TAIL_H else NORM_Q1
                    q2 = "gpsimd" if h >= TAIL_H else NORM_Q2
                    dq1 = nc.sync if q1 == "sync" else nc.gpsimd
                    dq2 = nc.sync if q2 == "sync" else nc.gpsimd
                    dq1.dma_start(scr[:], rec[DH : DH + 1, :])
                    bcast = normp.tile(
                        [DH, 512], BF16, tag="bcast", name=f"bcast{h}_{ih}"
                    )
                    bsrc = bass.AP(
                        tensor=scr.tensor, offset=scr.offset, ap=[[0, DH], [1, 512]]
                    )
                    dq2.dma_start(bcast[:], bsrc)
                    return cps, bcast

                def emit_mult(h, ih, cps, bcast):
                    nc.vector.tensor_tensor(
                        out=ctxp[h // 2][:, h % 2, ih * 512 : (ih + 1) * 512],
                        in0=cps[0:DH, :],
                        in1=bcast[:],
                        op=mybir.AluOpType.mult,
                    )

                # software-pipelined: head h's ctx/normalization is processed
                # as two independent [65,512] ih-rounds injected into head
                # h+1's exp stream (ctx+recip at j2/j4, the multiplies at
                # j6/j1 of the following head), so psum stays at 8 banks
                # with a 3-deep scores ring and the DVE interleaves norm ops
                # between exps instead of bursting.
                hs = {}
                rounds = {}
                for h in range(NH + 1):
                    cbs = {}
                    if 0 <= h - 1 < NH:
                        def _r0(hh=h - 1):
                            rounds[(hh, 0)] = emit_ctx_recip(hh, 0, hs[hh])
                        def _r1(hh=h - 1):
                            rounds[(hh, 1)] = emit_ctx_recip(hh, 1, hs[hh])
                        def _m0(hh=h - 1):
                            emit_mult(hh, 0, *rounds.pop((hh, 0)))
                        def _m1(hh=h - 1):
                            emit_mult(hh, 1, *rounds.pop((hh, 1)))
                        cbs[0] = _r0
                        cbs[2] = _r1
                        cbs[4] = _m0
                        cbs[6] = _m1
                    if h < NH:
                        hs[h] = emit_scores_exps(h, cbs)
                    else:
                        for j in sorted(cbs):
                            cbs[j]()

            # ---- phase C: out projection + residual + LayerNorm ----
            if ln_affine:
                lng_sb = small.tile([P, H], FP32, tag="lng")
                nc.sync.dma_start(lng_sb[:], lng)
                lnb_sb = small.tile([P, H], FP32, tag="lnb")
                nc.sync.dma_start(lnb_sb[:], lnb)

            ident_sb = small.tile([P, P], BF16, tag="ident")
            nc.sync.dma_start(ident_sb[:], ident)

            NSTAT = 2
            with (
                tc.tile_pool(name="psP", bufs=2, space="PSUM") as psP,
                tc.tile_pool(name="psO", bufs=2, space="PSUM") as psO,
                tc.tile_pool(name="xrp", bufs=ST) as xrp,
                tc.tile_pool(name="atp", bufs=ST) as atp,
            ):
                xrs = []
                for st in range(ST):
                    xr = xrp.tile([P, H], BF16, tag="xr", name=f"xr{st}")
                    nc.sync.dma_start(xr[:], xres[st * P : (st + 1) * P, :])
                    xrs.append(xr)

                # stage 1: hp0..6 partial, gated only by mult(13) — runs
                # during the last heads' norm chains; evacuated to bf16 SBUF
                atps = []
                def emit_partial(st):
                    ssl = slice(st * P, (st + 1) * P)
                    pp = psP.tile([P, H], FP32, tag="psP", name=f"pp{st}")
                    for nh in range(2):
                        for hp in range(PARTIAL_HPS):
                            nc.tensor.matmul(
                                pp[:, nh * 512 : (nh + 1) * 512],
                                lhsT=ctxp[hp][:, :, ssl],
                                rhs=wo_sb[:, 2 * hp : 2 * hp + 2, nh * 512 : (nh + 1) * 512],
                                start=(hp == 0),
                                stop=(hp == PARTIAL_HPS - 1),
                                perf_mode=DR,
                            )
                    ab = atp.tile([P, H], BF16, tag="atp", name=f"ab{st}")
                    nc.scalar.activation(
                        out=ab[:], in_=pp[:],
                        func=mybir.ActivationFunctionType.Copy,
                    )
                    # residual pre-add on the idle GpSimd engine; the finish
                    # stage then needs only one identity matmul per half
                    nc.gpsimd.tensor_tensor(
                        out=ab[:], in0=ab[:], in1=xrs[st][:],
                        op=mybir.AluOpType.add,
                    )
                    atps.append(ab)

                def emit_ln(st):
                    ssl = slice(st * P, (st + 1) * P)
                    att = psO.tile([P, H], FP32, tag="psO", name=f"att{st}")
                    for nh in range(2):
                        nsl = slice(nh * 512, (nh + 1) * 512)
                        for hp in range(PARTIAL_HPS, NH // 2):
                            nc.tensor.matmul(
                                att[:, nsl],
                                lhsT=ctxp[hp][:, :, ssl],
                                rhs=wo_sb[:, 2 * hp : 2 * hp + 2, nsl],
                                start=(hp == PARTIAL_HPS),
                                stop=False,
                                perf_mode=DR,
                            )
                        # add back the (partial + residual) sum
                        nc.tensor.matmul(
                            att[:, nsl],
                            lhsT=ident_sb[:],
                            rhs=atps[st][:, nsl],
                            start=False,
                            stop=True,
                        )
                    stats = lnp.tile([P, NSTAT, 6], FP32, tag="stats")
                    av = att[:].rearrange("p (n f) -> p n f", n=NSTAT)
                    for i in range(NSTAT):
                        nc.vector.bn_stats(out=stats[:, i, :], in_=av[:, i, :])
                    mv = lnp.tile([P, 2], FP32, tag="mv")
                    nc.vector.bn_aggr(out=mv[:], in_=stats[:])
                    rstd = lnp.tile([P, 1], FP32, tag="rstd")
                    nc.scalar.activation(
                        out=rstd[:],
                        in_=mv[:, 1:2],
                        func=mybir.ActivationFunctionType.Sqrt,
                        bias=eps_sb[:],
                    )
                    nc.vector.reciprocal(out=rstd[:], in_=rstd[:])
                    nbias = lnp.tile([P, 1], FP32, tag="nbias")
                    nc.vector.tensor_scalar(
                        out=nbias[:],
                        in0=mv[:, 0:1],
                        scalar1=rstd[:],
                        scalar2=-1.0,
                        op0=mybir.AluOpType.mult,
                        op1=mybir.AluOpType.mult,
                    )
                    t = lnp.tile([P, H], BF16, tag="t")
                    # (att - mean)*rstd as one ACT pass: att*rstd + (-mean*rstd)
                    nc.scalar.activation(
                        out=t[:], in_=att[:],
                        func=mybir.ActivationFunctionType.Identity,
                        scale=rstd[:, 0:1],
                        bias=nbias[:, 0:1],
                    )
                    if ln_affine:
                        nc.vector.tensor_tensor(
                            out=t[:], in0=t[:], in1=lng_sb[:], op=mybir.AluOpType.mult
                        )
                        nc.vector.tensor_tensor(
                            out=t[:], in0=t[:], in1=lnb_sb[:], op=mybir.AluOpType.add
                        )
                    nc.sync.dma_start(out[ssl, :], t[:])

                for st in range(ST):
                    emit_partial(st)
                for st in range(ST):
                    emit_ln(st)

    nc.compile()
    return nc


_prog_cache = {}


def _get_program(ln_affine=True):
    if ln_affine not in _prog_cache:
        _prog_cache[ln_affine] = _build_program(ln_affine=ln_affine)
    return _prog_cache[ln_affine]


def _prep_core_inputs(inputs, c):
    f32 = np.float32
    Wq = np.asarray(inputs["Wq"], f32)
    Wk = np.asarray(inputs["Wk"], f32)
    Wv = np.asarray(inputs["Wv"], f32)
    Wo = np.asarray(inputs["Wo"], f32)
    bq = np.asarray(inputs["bq"], f32)
    bv = np.asarray(inputs["bv"], f32)
    bo = np.asarray(inputs["bo"], f32)
    ln_g = np.asarray(inputs["ln_g"], f32)
    ln_b = np.asarray(inputs["ln_b"], f32)
    xq = np.asarray(inputs["query_tensors"][c], f32)
    xk = np.asarray(inputs["key_tensors"][c], f32)
    xv = np.asarray(inputs["value_tensors"][c], f32)
    mask = np.asarray(inputs["attention_mask"][c], f32).reshape(-1)[:S]

    bo_eff = bo + Wo @ bv

    def f8(x):
        return np.ascontiguousarray(x.astype(_nbf8))

    maskterm = ((1.0 - mask) * -10000.0).astype(f32)
    # keep the Schraudolph affine out of the int8 NaN band: anything below
    # exp(-12) is indistinguishable from 0 at fp8, so clamp to the
    # saturating -1e4 regime
    mclamp = np.where(maskterm < -12.0, -10000.0, maskterm)

    return {
        "xq8": f8(xq.T),
        "xk8": f8(xk.T),
        "xv8": f8(xv.T),
        "wq8": f8((Wq * W_SCALE).T),
        "wk8": f8((Wk * W_SCALE).T),
        "wv8": f8((Wv * W_SCALE).T),
        # (Wo*64).T is [c, o] with c = h*64+d; store as [d, h, o]
        "wo8": f8(
            (Wo * WO_SCALE).T.reshape(NH, DH, H).transpose(1, 0, 2)
        ),
        "bq": np.ascontiguousarray(bq.reshape(KO, P).T.astype(f32)),
        "maskb": np.ascontiguousarray(maskterm.reshape(ST, P).T),
        "sbias": np.ascontiguousarray(
            (EXPA * mclamp + 56.0 + EXPD).astype(f32).reshape(ST, P).T
        ),
        "xres": np.ascontiguousarray(
            (RES_SCALE * (xq + bo_eff[None, :])).astype(ml_dtypes.bfloat16)
        ),
        "ident": np.ascontiguousarray(np.eye(P, dtype=ml_dtypes.bfloat16)),
        "lng": np.ascontiguousarray(np.broadcast_to(ln_g, (P, H)).astype(f32)),
        "lnb": np.ascontiguousarray(np.broadcast_to(ln_b, (P, H)).astype(f32)),
    }


def kernel(**inputs) -> np.ndarray:
    ln_affine = not (
        np.all(np.asarray(inputs["ln_g"], np.float32) == 1.0)
        and np.all(np.asarray(inputs["ln_b"], np.float32) == 0.0)
    )
    nc = _get_program(ln_affine=ln_affine)
    in_maps = [_prep_core_inputs(inputs, c) for c in range(B)]
    if not ln_affine:
        for m in in_maps:
            m.pop("lng")
            m.pop("lnb")
    res = run_bass_kernel_spmd(nc, in_maps, core_ids=list(range(B)))
    out = np.stack([res.results[c]["out"] for c in range(B)], axis=0)
    return out.astype(np.float32)


if __name__ == "__main__":
    nc = _build_program(ln_affine=False)
    print("program built ok")
